# revision 1
# baseline (speedup 1.0000x reference)
"""Trainium2 Bass kernel for KANCell: relu(sum(relu(x))) over 2**25 fp32.

Data-parallel sharded reduction: the flat vector is split across 8
NeuronCores. Each core streams its 16 MiB shard HBM->SBUF in 2 MiB tiles
and fuses relu + per-partition partial sum in a single ScalarE activation
(accum_out) per tile, overlapped with the next tile's DMA. Per-core
partials come back to the host, which does the final (tiny) sum + ReLU.

Raw bass (no TileContext): all input DMAs share one semaphore so the
instruction-level sync-wait counts stay within walrus's limits.
"""

import numpy as np

N = 33554432  # 2**25
N_CORES = 8
PER_CORE = N // N_CORES  # 4194304 elements, 16 MiB fp32
P = 128  # SBUF partitions
F = 1024  # free-dim elements per tile -> [128, 1024] fp32 = 512 KiB
N_TILES = PER_CORE // (P * F)  # 32

_CACHED = {}


def _build_nc_iters(n_iters: int, f: int = F):
    """Build the per-core program with the pipeline body repeated n_iters
    times (n_iters>1 is used only for wall-clock slope benchmarking)."""
    key = (n_iters, f)
    if key in _CACHED:
        return _CACHED[key]

    import concourse.bass as bass
    import concourse.mybir as mybir

    n_tiles = PER_CORE // (P * f)
    nc = bass.Bass()

    x = nc.declare_dram_parameter("x", [PER_CORE], mybir.dt.float32, isOutput=False)
    out = nc.declare_dram_parameter(
        "partials", [P, n_tiles], mybir.dt.float32, isOutput=True
    )
    xv = x.rearrange("(n p f) -> n p f", p=P, f=f)

    from contextlib import ExitStack

    with ExitStack() as ctx:
        buf = ctx.enter_context(
            nc.sbuf_tensor([P, n_tiles * f], mybir.dt.float32)
        )
        accs = ctx.enter_context(nc.sbuf_tensor([P, n_tiles], mybir.dt.float32))
        # One completion semaphore per tile: sem_i >= 16 iff DMA i fully
        # landed. A single shared semaphore is NOT safe here — SDMA engines
        # skew, so a count of 16*(i+1) can be reached before DMA i+1's last
        # byte lands (observed as undercounted sums at 32 tiles in flight).
        in_sems = [
            ctx.enter_context(nc.semaphore(name=f"in_sem_{i}"))
            for i in range(n_tiles)
        ]
        act_sem = ctx.enter_context(nc.semaphore())
        out_sem = ctx.enter_context(nc.semaphore())
        block = ctx.enter_context(nc.Block())

        @block.sync
        def _(sync):
            for j in range(n_iters):
                for i in range(n_tiles):
                    if j > 0:
                        # WAR: iter j's DMA into tile i must wait for
                        # iter j-1's ACT on tile i to have consumed it.
                        sync.wait_ge(act_sem, (j - 1) * n_tiles + i + 1)
                    sync.dma_start(
                        out=buf[:, i * f : (i + 1) * f], in_=xv[i]
                    ).then_inc(in_sems[i], 16)
                sync.wait_ge(act_sem, (j + 1) * n_tiles)
                sync.dma_start(out=out[:], in_=accs[:]).then_inc(out_sem, 16)
            sync.wait_ge(out_sem, 16 * n_iters)

        @block.scalar
        def _(scalar):
            for j in range(n_iters):
                if j > 0:
                    # WAR: accs is re-written each iter; previous gather
                    # DMA must have read it.
                    scalar.wait_ge(out_sem, 16 * j)
                for i in range(n_tiles):
                    scalar.wait_ge(in_sems[i], 16 * (j + 1))
                    # in-place relu; per-partition tile sum -> accs[:, i]
                    nc.scalar.activation(
                        buf[:, i * f : (i + 1) * f],
                        buf[:, i * f : (i + 1) * f],
                        mybir.ActivationFunctionType.Relu,
                        accum_out=accs[:, i : i + 1],
                    ).then_inc(act_sem, 1)

    _CACHED[key] = nc
    return nc


def _build_nc():
    return _build_nc_iters(1)


def kernel(x: np.ndarray) -> np.ndarray:
    from concourse.bass_utils import run_bass_kernel_spmd

    nc = _build_nc()

    x = np.ascontiguousarray(np.asarray(x, dtype=np.float32).reshape(-1))
    shards = x.reshape(N_CORES, PER_CORE)
    in_maps = [{"x": shards[i]} for i in range(N_CORES)]
    res = run_bass_kernel_spmd(nc, in_maps, list(range(N_CORES)))

    partials = np.stack([r["partials"] for r in res.results])  # [8, P, n_tiles]
    total = partials.astype(np.float64).sum()
    return np.asarray(max(total, 0.0), dtype=np.float32)



# revision 6
# speedup vs baseline: 3.0803x; 3.0803x over previous
"""Trainium2 Bass kernel for KANCell: relu(sum(relu(x))) over 2**25 fp32.

Data-parallel sharded reduction across 8 NeuronCores; each core reduces a
16 MiB shard. Within a core, the work is spread over four engine queues so
their costs overlap instead of serializing on one DMA stream:

  - gpsimd (Pool/SWDGE): casting DMAs fp32(HBM) -> bf16(SBUF) for ~half the
    shard. The cast halves the SBUF-side bytes, and bf16 tiles hit DVE's
    4x perf mode downstream.
  - sync (SP) + scalar (Act) HWDGE queues: plain fp32 DMAs for the rest;
    the Act engine additionally relu+accums one fp32 chunk at the end of
    its queue (activation with accum_out).
  - vector (DVE): fused relu+sum via tensor_scalar(max 0, accum add) over
    all remaining tiles, in DMA arrival order (bf16 tiles at the 4x rate,
    fp32 tiles at the 2x SBUF rate).

Per-core partials [128, n_accs] come back to the host, which does the
final (tiny) cross-core sum + ReLU. Tile sizes were tuned (simulated
annealing against the cost-model timeline) so the three DMA queues finish
together and the consumers (DVE / Act) drain right behind the last
arrivals.

Raw bass (no TileContext). Each DMA gets its own semaphore (SDMA engines
skew; a shared counting semaphore can be observed at 16*(i+1) before DMA
i+1's last byte lands).
"""

import numpy as np

N = 33554432  # 2**25
N_CORES = 8
PER_CORE = N // N_CORES  # 4194304 elements, 16 MiB fp32
P = 128  # SBUF partitions
COLS = PER_CORE // P  # 32768 fp32 columns of [128]

# Tuned tile schedules (columns). pool tiles are cast to bf16.
POOL_TILES = [1722, 1559, 1789, 3221, 1067, 1859, 1292, 972, 2057, 1181, 659]
SP_TILES = [306, 811, 1097, 454, 518, 2944, 672, 726, 673]
ACT_TILES = [613, 789, 1349, 382, 352, 495, 1198, 802, 497, 712]
SP_RELU_IDX = (5,)  # sp tile relu+accum'd by the Act engine, not DVE

# DVE consumption order (queue, lo_tile, hi_tile), sorted by modeled DMA
# arrival time so the vector queue never head-of-line blocks.
DVE_PLAN = [
    ("sp", 0, 0), ("act", 0, 0), ("pool", 0, 0), ("sp", 1, 1),
    ("act", 1, 1), ("pool", 1, 1), ("sp", 2, 2), ("pool", 2, 2),
    ("sp", 3, 3), ("act", 2, 3), ("sp", 4, 4), ("act", 4, 5),
    ("pool", 3, 3), ("pool", 4, 4), ("act", 6, 6), ("pool", 5, 5),
    ("act", 7, 7), ("pool", 6, 6), ("act", 8, 8), ("pool", 7, 7),
    ("sp", 6, 6), ("act", 9, 9), ("sp", 7, 7), ("pool", 8, 8),
    ("sp", 8, 8), ("pool", 9, 9), ("pool", 10, 10),
]

assert sum(POOL_TILES) + sum(SP_TILES) + sum(ACT_TILES) == COLS

_CACHED = {}


def _build_nc():
    key = "main"
    if key in _CACHED:
        return _CACHED[key]

    import concourse.bass as bass
    import concourse.mybir as mybir
    from contextlib import ExitStack

    pool_cols = sum(POOL_TILES)
    fp32_cols = sum(SP_TILES) + sum(ACT_TILES)
    n_dve = len(DVE_PLAN)
    n_out = n_dve + 1  # + act relu accum
    act_acc_col = n_dve

    nc = bass.Bass()
    x = nc.declare_dram_parameter("x", [PER_CORE], mybir.dt.float32, isOutput=False)
    out = nc.declare_dram_parameter(
        "partials", [P, n_out], mybir.dt.float32, isOutput=True
    )
    xv = x.rearrange("(p f) -> p f", p=P, f=COLS)

    def bounds(tiles, base):
        res, o = [], base
        for c in tiles:
            res.append((o, o + c))
            o += c
        return res

    pool_b = bounds(POOL_TILES, 0)
    sp_b = bounds(SP_TILES, pool_cols)
    act_b = bounds(ACT_TILES, pool_cols + sum(SP_TILES))

    with ExitStack() as ctx:
        bbuf = ctx.enter_context(nc.sbuf_tensor([P, pool_cols], mybir.dt.bfloat16))
        fbuf = ctx.enter_context(nc.sbuf_tensor([P, fp32_cols], mybir.dt.float32))
        accs = ctx.enter_context(nc.sbuf_tensor([P, n_out], mybir.dt.float32))

        p_sems = [
            ctx.enter_context(nc.semaphore(name=f"p{i}"))
            for i in range(len(POOL_TILES))
        ]
        s_sems = [
            ctx.enter_context(nc.semaphore(name=f"s{i}"))
            for i in range(len(SP_TILES))
        ]
        a_sems = [
            ctx.enter_context(nc.semaphore(name=f"a{i}"))
            for i in range(len(ACT_TILES))
        ]
        rsem = ctx.enter_context(nc.semaphore(name="rsem"))
        done = ctx.enter_context(nc.semaphore(name="done"))
        block = ctx.enter_context(nc.Block())

        n_sigs = n_dve + 1

        @block.gpsimd
        def _(gp):
            # casting DMAs: fp32 HBM -> bf16 SBUF (SWDGE converts on the fly)
            for i, (lo, hi) in enumerate(pool_b):
                gp.dma_start(out=bbuf[:, lo:hi], in_=xv[:, lo:hi]).then_inc(
                    p_sems[i], 16
                )

        @block.sync
        def _(sync):
            for i, (lo, hi) in enumerate(sp_b):
                sync.dma_start(
                    out=fbuf[:, lo - pool_cols : hi - pool_cols], in_=xv[:, lo:hi]
                ).then_inc(s_sems[i], 16)
            sync.wait_ge(rsem, n_sigs)
            sync.dma_start(out=out[:], in_=accs[:]).then_inc(done, 16)
            sync.wait_ge(done, 16)

        @block.scalar
        def _(scalar):
            for i, (lo, hi) in enumerate(act_b):
                scalar.dma_start(
                    out=fbuf[:, lo - pool_cols : hi - pool_cols], in_=xv[:, lo:hi]
                ).then_inc(a_sems[i], 16)
            # relu+accum one sp chunk on the Act engine
            for i in SP_RELU_IDX:
                scalar.wait_ge(s_sems[i], 16)
            lo = sp_b[SP_RELU_IDX[0]][0] - pool_cols
            hi = sp_b[SP_RELU_IDX[-1]][1] - pool_cols
            scalar.activation(
                fbuf[:, lo:hi],
                fbuf[:, lo:hi],
                mybir.ActivationFunctionType.Relu,
                accum_out=accs[:, act_acc_col : act_acc_col + 1],
            ).then_inc(rsem, 1)

        @block.vector
        def _(vector):
            k = 0
            for q, lo_i, hi_i in DVE_PLAN:
                if q == "pool":
                    b, sems = pool_b, p_sems
                elif q == "sp":
                    b, sems = sp_b, s_sems
                else:
                    b, sems = act_b, a_sems
                for t in range(lo_i, hi_i + 1):
                    vector.wait_ge(sems[t], 16)
                lo, hi = b[lo_i][0], b[hi_i][1]
                src = (
                    bbuf[:, lo:hi]
                    if q == "pool"
                    else fbuf[:, lo - pool_cols : hi - pool_cols]
                )
                vector.tensor_scalar(
                    out=src,
                    in0=src,
                    scalar1=0.0,
                    scalar2=None,
                    op0=mybir.AluOpType.max,
                    op1=mybir.AluOpType.add,
                    accum_out=accs[:, k : k + 1],
                ).then_inc(rsem, 1)
                k += 1

    _CACHED[key] = nc
    return nc


def kernel(x: np.ndarray) -> np.ndarray:
    from concourse.bass_utils import run_bass_kernel_spmd

    nc = _build_nc()

    x = np.ascontiguousarray(np.asarray(x, dtype=np.float32).reshape(-1))
    shards = x.reshape(N_CORES, PER_CORE)
    in_maps = [{"x": shards[i]} for i in range(N_CORES)]
    res = run_bass_kernel_spmd(nc, in_maps, list(range(N_CORES)))

    partials = np.stack([r["partials"] for r in res.results])  # [8, P, n_out]
    total = partials.astype(np.float64).sum()
    return np.asarray(max(total, 0.0), dtype=np.float32)


# revision 7
# speedup vs baseline: 3.0975x; 1.0056x over previous
"""Trainium2 Bass kernel for KANCell: relu(sum(relu(x))) over 2**25 fp32.

Data-parallel sharded reduction across 8 NeuronCores; each core reduces a
16 MiB shard. Within a core, the work is spread over four engine queues so
their costs overlap instead of serializing on one DMA stream:

  - gpsimd (Pool/SWDGE): casting DMAs fp32(HBM) -> bf16(SBUF) for ~half the
    shard. The cast halves the SBUF-side bytes, and bf16 tiles hit DVE's
    4x perf mode downstream.
  - sync (SP) + scalar (Act) HWDGE queues: plain fp32 DMAs for the rest;
    the Act engine additionally relu+accums one fp32 chunk at the end of
    its queue (activation with accum_out).
  - vector (DVE): fused relu+sum via tensor_scalar(max 0, accum add) over
    all remaining tiles, in DMA arrival order (bf16 tiles at the 4x rate,
    fp32 tiles at the 2x SBUF rate).

Per-core partials [128, n_accs] come back to the host, which does the
final (tiny) cross-core sum + ReLU. Tile sizes were tuned (simulated
annealing against the cost-model timeline) so the three DMA queues finish
together and the consumers (DVE / Act) drain right behind the last
arrivals.

Raw bass (no TileContext). Each DMA gets its own semaphore (SDMA engines
skew; a shared counting semaphore can be observed at 16*(i+1) before DMA
i+1's last byte lands).
"""

import numpy as np

N = 33554432  # 2**25
N_CORES = 8
PER_CORE = N // N_CORES  # 4194304 elements, 16 MiB fp32
P = 128  # SBUF partitions
COLS = PER_CORE // P  # 32768 fp32 columns of [128]

# Tuned tile schedules (columns). pool tiles are cast to bf16.
POOL_TILES = [1722, 1559, 1789, 3221, 1067, 1859, 1292, 972, 2057, 1181, 659]
SP_TILES = [306, 811, 1097, 454, 518, 2944, 672, 726, 673]
ACT_TILES = [613, 789, 1349, 382, 352, 495, 1198, 802, 497, 712]
SP_RELU_IDX = (5,)  # sp tile relu+accum'd by the Act engine, not DVE

# DVE consumption order (queue, lo_tile, hi_tile), sorted by modeled DMA
# arrival time so the vector queue never head-of-line blocks.
DVE_PLAN = [
    ("sp", 0, 0), ("act", 0, 0), ("pool", 0, 0), ("sp", 1, 1),
    ("act", 1, 1), ("pool", 1, 1), ("sp", 2, 2), ("pool", 2, 2),
    ("sp", 3, 3), ("act", 2, 3), ("sp", 4, 4), ("act", 4, 5),
    ("pool", 3, 3), ("pool", 4, 4), ("act", 6, 6), ("pool", 5, 5),
    ("act", 7, 7), ("pool", 6, 6), ("act", 8, 8), ("pool", 7, 7),
    ("sp", 6, 6), ("act", 9, 9), ("sp", 7, 7), ("pool", 8, 8),
    ("sp", 8, 8), ("pool", 9, 9), ("pool", 10, 10),
]

assert sum(POOL_TILES) + sum(SP_TILES) + sum(ACT_TILES) == COLS

_CACHED = {}


def _build_nc():
    key = "main"
    if key in _CACHED:
        return _CACHED[key]

    import concourse.bass as bass
    import concourse.mybir as mybir
    from contextlib import ExitStack

    pool_cols = sum(POOL_TILES)
    fp32_cols = sum(SP_TILES) + sum(ACT_TILES)
    n_dve = len(DVE_PLAN)
    n_out = n_dve + 1  # + act relu accum
    act_acc_col = n_dve

    nc = bass.Bass()
    x = nc.declare_dram_parameter("x", [PER_CORE], mybir.dt.float32, isOutput=False)
    out = nc.declare_dram_parameter(
        "partials", [P, n_out], mybir.dt.float32, isOutput=True
    )
    xv = x.rearrange("(p f) -> p f", p=P, f=COLS)

    def bounds(tiles, base):
        res, o = [], base
        for c in tiles:
            res.append((o, o + c))
            o += c
        return res

    pool_b = bounds(POOL_TILES, 0)
    sp_b = bounds(SP_TILES, pool_cols)
    act_b = bounds(ACT_TILES, pool_cols + sum(SP_TILES))

    with ExitStack() as ctx:
        bbuf = ctx.enter_context(nc.sbuf_tensor([P, pool_cols], mybir.dt.bfloat16))
        fbuf = ctx.enter_context(nc.sbuf_tensor([P, fp32_cols], mybir.dt.float32))
        accs = ctx.enter_context(nc.sbuf_tensor([P, n_out], mybir.dt.float32))

        p_sems = [
            ctx.enter_context(nc.semaphore(name=f"p{i}"))
            for i in range(len(POOL_TILES))
        ]
        s_sems = [
            ctx.enter_context(nc.semaphore(name=f"s{i}"))
            for i in range(len(SP_TILES))
        ]
        a_sems = [
            ctx.enter_context(nc.semaphore(name=f"a{i}"))
            for i in range(len(ACT_TILES))
        ]
        rsem = ctx.enter_context(nc.semaphore(name="rsem"))
        done = ctx.enter_context(nc.semaphore(name="done"))
        block = ctx.enter_context(nc.Block())

        n_sigs = n_dve + 1

        @block.gpsimd
        def _(gp):
            # casting DMAs: fp32 HBM -> bf16 SBUF (SWDGE converts on the fly)
            for i, (lo, hi) in enumerate(pool_b):
                gp.dma_start(out=bbuf[:, lo:hi], in_=xv[:, lo:hi]).then_inc(
                    p_sems[i], 16
                )

        @block.sync
        def _(sync):
            for i, (lo, hi) in enumerate(sp_b):
                sync.dma_start(
                    out=fbuf[:, lo - pool_cols : hi - pool_cols], in_=xv[:, lo:hi]
                ).then_inc(s_sems[i], 16)

        @block.scalar
        def _(scalar):
            for i, (lo, hi) in enumerate(act_b):
                scalar.dma_start(
                    out=fbuf[:, lo - pool_cols : hi - pool_cols], in_=xv[:, lo:hi]
                ).then_inc(a_sems[i], 16)
            # relu+accum one sp chunk on the Act engine
            for i in SP_RELU_IDX:
                scalar.wait_ge(s_sems[i], 16)
            lo = sp_b[SP_RELU_IDX[0]][0] - pool_cols
            hi = sp_b[SP_RELU_IDX[-1]][1] - pool_cols
            scalar.activation(
                fbuf[:, lo:hi],
                fbuf[:, lo:hi],
                mybir.ActivationFunctionType.Relu,
                accum_out=accs[:, act_acc_col : act_acc_col + 1],
            ).then_inc(rsem, 1)
            # the Act engine usually finishes last, so it issues the gather of
            # the partials itself: no cross-engine sem hop before the out DMA
            scalar.wait_ge(rsem, n_sigs)
            scalar.dma_start(out=out[:], in_=accs[:]).then_inc(done, 16)
            scalar.wait_ge(done, 16)

        @block.vector
        def _(vector):
            k = 0
            for q, lo_i, hi_i in DVE_PLAN:
                if q == "pool":
                    b, sems = pool_b, p_sems
                elif q == "sp":
                    b, sems = sp_b, s_sems
                else:
                    b, sems = act_b, a_sems
                for t in range(lo_i, hi_i + 1):
                    vector.wait_ge(sems[t], 16)
                lo, hi = b[lo_i][0], b[hi_i][1]
                src = (
                    bbuf[:, lo:hi]
                    if q == "pool"
                    else fbuf[:, lo - pool_cols : hi - pool_cols]
                )
                vector.tensor_scalar(
                    out=src,
                    in0=src,
                    scalar1=0.0,
                    scalar2=None,
                    op0=mybir.AluOpType.max,
                    op1=mybir.AluOpType.add,
                    accum_out=accs[:, k : k + 1],
                ).then_inc(rsem, 1)
                k += 1

    _CACHED[key] = nc
    return nc


def kernel(x: np.ndarray) -> np.ndarray:
    from concourse.bass_utils import run_bass_kernel_spmd

    nc = _build_nc()

    x = np.ascontiguousarray(np.asarray(x, dtype=np.float32).reshape(-1))
    shards = x.reshape(N_CORES, PER_CORE)
    in_maps = [{"x": shards[i]} for i in range(N_CORES)]
    res = run_bass_kernel_spmd(nc, in_maps, list(range(N_CORES)))

    partials = np.stack([r["partials"] for r in res.results])  # [8, P, n_out]
    total = partials.astype(np.float64).sum()
    return np.asarray(max(total, 0.0), dtype=np.float32)
